# revision 2
# baseline (speedup 1.0000x reference)
"""GCN layer kernel for 8 Trainium2 NeuronCores.

out[i] = sum over edges (i<-j) of a_ij * (x @ W^T)[j]
       = ((A @ x) @ W^T)[i]

Device strategy (per core):
  - x is replicated (fp16 copy for gather bandwidth); its row space is split
    into NCHUNK ranges of CH rows so dma_gather's int16 indices can address
    them. Edges are partitioned by src-chunk, then sorted by dst and cut
    into UNIFORM blocks of J*128 = 256 edge slots. Because a block has at
    most 256 edges it covers <= SW=256 distinct dst nodes (slots) by
    construction, so blocks are FULL (no gather padding). Every core gets
    the same number of blocks per chunk (dummy all-pad blocks equalize), so
    one program serves all cores (SPMD).
  - One dma_gather per GPB blocks (2048 rows, 256B each) streams x[src] into
    SBUF: xg[p, s*128+f] = x[src(stream slot s*128+p), f].
  - Per 128-edge chunk c of a block, DVE builds
    S_c[e, slot] = val[e] * (dstloc[e] == slot)  (is_equal + mult),
    and PE accumulates yT[feat, slot] += Xg_c^T @ S_c in PSUM (J matmuls).
  - Epilogue: out[slot, :] = yT^T @ W^T via NH matmuls (slot halves), ACT
    copies PSUM->SBUF staging (fp16), one output DMA per OGRP blocks.
  - Host un-permutes slot rows back to node ids, summing duplicates
    (nodes may appear in blocks of several chunks / straddle blocks).
"""

import numpy as np

P = 128
D = 128
SW = 256  # dst slots per block (PSUM free dim of yT)
J = 2  # 128-edge chunks per block; block = J*128 = 256 edge slots
BLK = J * P
GATH = 2048  # rows per dma_gather
GPB = GATH // (J * P)  # blocks per gather
NH = SW // D  # slot halves per block epilogue
CH = 25600  # x rows per chunk (int16-addressable)
N_CORES = 8
OGRP = 4  # blocks per output DMA
MGS = 16  # blocks per metadata group tile
USE_FP16 = True


def _build_program(chunk_rows, B_g, n_nodes, n_iters=1, variant="full"):
    """chunk_rows[g] = rows in x-chunk g; B_g[g] = blocks per core for chunk g.

    n_iters > 1 wraps the whole body in a For_i loop (for timing only)."""
    import concourse.bass as bass
    import concourse.mybir as mybir
    import concourse.tile as tile
    from concourse import bacc

    f32 = mybir.dt.float32
    i16 = mybir.dt.int16
    fx = mybir.dt.float16 if USE_FP16 else mybir.dt.float32

    NBt = sum(B_g)
    NG = NBt // GPB
    nchunk = len(B_g)
    boff = np.concatenate([[0], np.cumsum(B_g)]).astype(int)
    qoff = np.concatenate([[0], np.cumsum([b // GPB for b in B_g])]).astype(int)

    nc = bacc.Bacc(
        "TRN2",
        target_bir_lowering=False,
        debug=False,
        enable_asserts=True,
        num_devices=N_CORES,
    )
    x_d = nc.dram_tensor("x", [n_nodes, D], fx, kind="ExternalInput")
    idx_d = nc.dram_tensor("idx", [NG, P, GATH // 16], i16, kind="ExternalInput")
    meta_d = nc.dram_tensor("meta", [P, NBt * 2 * J], fx, kind="ExternalInput")
    wt_d = nc.dram_tensor("wt", [D, D], fx, kind="ExternalInput")
    iota_d = nc.dram_tensor("iota", [P, J * SW], fx, kind="ExternalInput")
    out_d = nc.dram_tensor(
        "out", [NBt // OGRP, P, OGRP * NH * D], fx, kind="ExternalOutput"
    )

    nmg = (NBt + MGS - 1) // MGS

    with tile.TileContext(nc) as tc:
        with (
            tc.tile_pool(name="const", bufs=1) as cpool,
            tc.tile_pool(name="idx", bufs=6) as idxpool,
            tc.tile_pool(name="xg", bufs=6) as xgpool,
            tc.tile_pool(name="s", bufs=8) as spool,
            tc.tile_pool(name="y", bufs=3) as ypool,
            tc.tile_pool(name="ostage", bufs=3) as opool,
            tc.tile_pool(name="scratch", bufs=1) as scpool,
            tc.tile_pool(name="ps_y", bufs=3, space="PSUM") as pspool,
            tc.tile_pool(name="ps_o", bufs=3, space="PSUM") as ps2pool,
        ):
            wt_t = cpool.tile([D, D], fx)
            nc.sync.dma_start(out=wt_t[:], in_=wt_d[:])
            iota_t = cpool.tile([P, J * SW], fx)
            nc.sync.dma_start(out=iota_t[:], in_=iota_d[:])

            meta_g = []
            for mg in range(nmg):
                nb = min(MGS, NBt - mg * MGS)
                mt = cpool.tile([P, nb * 2 * J], fx, tag=f"meta{mg}")
                nc.sync.dma_start(
                    out=mt[:],
                    in_=meta_d[:, mg * MGS * 2 * J : (mg * MGS + nb) * 2 * J],
                )
                meta_g.append(mt)

            # Absorb startup-DMA semaphores into the DVE in-order stream.
            sc = scpool.tile([P, 2], fx)
            nc.vector.tensor_copy(sc[:, 0:1], iota_t[:, 0:1])
            nc.vector.tensor_copy(sc[:, 1:2], iota_t[:, 1:2])
            sc2 = scpool.tile([P, 1], fx)
            nc.vector.tensor_copy(sc2[:], meta_g[0][:, 0:1])

            import contextlib

            loop_cm = (
                tc.For_i(0, n_iters, 1) if n_iters > 1 else contextlib.nullcontext()
            )
            with loop_cm:
                _body(nc, tc, mybir, bass, B_g, boff, qoff, chunk_rows,
                      meta_g, iota_t, wt_t,
                      idxpool, xgpool, spool, ypool, opool, pspool, ps2pool,
                      x_d, idx_d, out_d, fx, variant)

    nc.compile()
    return nc


def _body(nc, tc, mybir, bass, B_g, boff, qoff, chunk_rows, meta_g, iota_t, wt_t,
          idxpool, xgpool, spool, ypool, opool, pspool, ps2pool, x_d, idx_d, out_d, fx,
          variant="full"):
    do_gather = variant in ("full", "gather", "gather_q4")
    do_compute = variant in ("full", "compute", "compute_nots", "compute_noact")
    do_ts = variant not in ("compute_nots",)
    do_act = variant not in ("compute_noact",)
    qrr = variant == "gather_q4"
    f32 = mybir.dt.float32
    i16 = mybir.dt.int16
    nchunk = len(B_g)
    cur_xg = None
    for g in range(nchunk):
        for lb in range(B_g[g]):
            bi = int(boff[g]) + lb
            if lb % GPB == 0 and do_gather:
                q = int(qoff[g]) + lb // GPB
                idx_t = idxpool.tile([P, GATH // 16], i16, tag="idx")
                nc.sync.dma_start(out=idx_t[:], in_=idx_d[q])
                xg = xgpool.tile([P, (GATH // P) * D], fx, tag="xg")
                nc.gpsimd.dma_gather(
                    out_ap=xg[:].rearrange("p (j e) -> p j e", e=D),
                    in_ap=x_d[g * CH : g * CH + chunk_rows[g], :],
                    idxs_ap=idx_t[:],
                    num_idxs=GATH,
                    num_idxs_reg=GATH,
                    elem_size=D,
                    single_packet=False,
                    queue_num=(q % 4) if qrr else 0,
                )
                cur_xg = xg
            elif lb % GPB == 0 and cur_xg is None:
                cur_xg = xgpool.tile([P, (GATH // P) * D], fx, tag="xg")
                nc.vector.memset(cur_xg[:], 0)

            if not do_compute:
                continue
            yT_ps = pspool.tile([D, SW], mybir.dt.float32, space="PSUM")
            mt = meta_g[bi // MGS]
            mo = (bi % MGS) * 2 * J
            s4 = spool.tile([P, J * SW], fx, tag="s")
            if do_ts:
                dst_b = mt[:, mo : mo + J].to_broadcast([P, J, SW])
                val_b = mt[:, mo + J : mo + 2 * J].to_broadcast([P, J, SW])
                s4v = s4[:].rearrange("p (c j) -> p c j", j=SW)
                nc.vector.tensor_tensor(
                    out=s4v, in0=iota_t[:].rearrange("p (c j) -> p c j", j=SW),
                    in1=dst_b, op=mybir.AluOpType.is_equal,
                )
                nc.vector.tensor_tensor(
                    out=s4v, in0=s4v, in1=val_b, op=mybir.AluOpType.mult,
                )
            else:
                nc.vector.memset(s4[:], 0)
            for c in range(J):
                s = (lb % GPB) * J + c  # stream chunk within the gather
                nc.tensor.matmul(
                    out=yT_ps[:],
                    lhsT=cur_xg[:, s * D : (s + 1) * D],
                    rhs=s4[:, c * SW : (c + 1) * SW],
                    start=(c == 0),
                    stop=(c == J - 1),
                )

            yT_sb = ypool.tile([D, SW], fx, tag="yT")
            if do_act:
                nc.scalar.copy(yT_sb[:], yT_ps[:])
            out_ps = ps2pool.tile([P, NH * D], mybir.dt.float32, space="PSUM")
            for h in range(NH):
                nc.tensor.matmul(
                    out=out_ps[:, h * D : (h + 1) * D],
                    lhsT=yT_sb[:, h * D : (h + 1) * D],
                    rhs=wt_t[:],
                    start=True,
                    stop=True,
                )
            if bi % OGRP == 0:
                stage = opool.tile([P, OGRP * NH * D], fx, tag="stage")
            if do_act:
                nc.scalar.copy(
                    stage[
                        :,
                        (bi % OGRP) * NH * D : (bi % OGRP + 1) * NH * D,
                    ],
                    out_ps[:],
                )
                if bi % OGRP == OGRP - 1:
                    nc.sync.dma_start(out=out_d[bi // OGRP], in_=stage[:])


def _preprocess(dst, src, vals, n_nodes):
    """Build per-core device arrays.

    Returns (idx_arr[NC,NG,P,GATH//16] i16, meta_arr[NC,P,NBt*2J] fx,
             slot_ids[NC,NBt,SW] i64, chunk_rows, B_g).
    """
    fdt = np.float16 if USE_FP16 else np.float32
    nchunk = (n_nodes + CH - 1) // CH
    chunk_rows = [min(CH, n_nodes - g * CH) for g in range(nchunk)]
    chunk_of = src // CH

    # per chunk: globally sort by (dst), cut into blocks
    blocks = []  # list per chunk of (dg, sg, vg, cum, blist)
    for g in range(nchunk):
        m = chunk_of == g
        dg = dst[m]
        sg = (src[m] - g * CH).astype(np.int16)
        vg = vals[m]
        order = np.argsort(dg, kind="stable")
        dg, sg, vg = dg[order], sg[order], vg[order]
        Eg = dg.shape[0]
        blist = []
        if Eg:
            firstocc = np.empty(Eg, dtype=bool)
            firstocc[0] = True
            firstocc[1:] = dg[1:] != dg[:-1]
            cum = np.cumsum(firstocc)
            a = 0
            while a < Eg:
                j = np.searchsorted(cum, cum[a] + SW - 1, side="right") - 1
                b = min(a + BLK, j + 1, Eg)
                blist.append((a, b, cum[a]))
                a = b
        blocks.append((dg, sg, vg, cum if Eg else None, blist))

    nb_g = [len(bl[4]) for bl in blocks]
    B_g = [-(-n // N_CORES) for n in nb_g]  # ceil
    B_g = [-(-b // GPB) * GPB for b in B_g]  # multiple of blocks-per-gather
    if sum(B_g) % OGRP != 0:
        B_g[0] += GPB
    NBt = sum(B_g)
    NG = NBt // GPB

    idx_arr = np.zeros((N_CORES, NG, P, GATH // 16), dtype=np.int16)
    meta_arr = np.zeros((N_CORES, P, NBt * 2 * J), dtype=fdt)
    slot_ids = np.full((N_CORES, NBt, SW), -1, dtype=np.int64)

    boff = np.concatenate([[0], np.cumsum(B_g)]).astype(int)
    qoff = np.concatenate([[0], np.cumsum([b // GPB for b in B_g])]).astype(int)

    for g in range(nchunk):
        dg, sg, vg, cum, blist = blocks[g]
        for k, (a, b, cum_a) in enumerate(blist):
            core, lb = k % N_CORES, k // N_CORES
            bi = int(boff[g]) + lb
            n = b - a
            f = np.arange(n)
            c, p = f // P, f % P
            rank = (cum[a:b] - cum_a).astype(np.int64)
            # metadata: dstloc & vals at [p, bi*2J + c] / [p, bi*2J + J + c]
            meta_arr[core, p, bi * 2 * J + c] = rank.astype(fdt)
            meta_arr[core, p, bi * 2 * J + J + c] = vg[a:b].astype(fdt)
            slot_ids[core, bi, rank] = dg[a:b]
            # gather indices: stream slot = lb*BLK + f; q = slot//GATH,
            # i = slot%GATH, wrapped at [16*grp + i%16, i//16]
            slot = lb * BLK + f
            q = int(qoff[g]) + lb // GPB
            i = slot % GATH
            idx_arr[core, q, i % 16, i // 16] = sg[a:b]
    # replicate idx rows across the 8 16-partition groups
    idx_arr = np.tile(idx_arr[:, :, :16, :], (1, 1, 8, 1))
    return idx_arr, meta_arr, slot_ids, chunk_rows, B_g


def make_in_maps(x, weight, edge_index, edge_vals, num_nodes):
    """Host preprocessing -> (in_maps, slot_ids, chunk_rows, B_g)."""
    fdt = np.float16 if USE_FP16 else np.float32
    x = np.asarray(x, dtype=np.float32)
    weight = np.asarray(weight, dtype=np.float32)
    dst = np.asarray(edge_index[0], dtype=np.int64)
    src = np.asarray(edge_index[1], dtype=np.int64)
    vals = np.asarray(edge_vals, dtype=np.float32)
    N = int(num_nodes)

    idx_arr, meta_arr, slot_ids, chunk_rows, B_g = _preprocess(dst, src, vals, N)

    xg = np.ascontiguousarray(x.astype(fdt))
    wt = np.ascontiguousarray(weight.T.astype(fdt))
    iota = np.tile(np.tile(np.arange(SW, dtype=fdt), J), (P, 1))

    in_maps = [
        {
            "x": xg,
            "idx": idx_arr[k],
            "meta": meta_arr[k],
            "wt": wt,
            "iota": iota,
        }
        for k in range(N_CORES)
    ]
    return in_maps, slot_ids, chunk_rows, B_g


def combine_output(results, slot_ids, B_g, n_nodes):
    """Host-side: un-permute slot rows back to node ids, summing dups."""
    NBt = sum(B_g)
    out = np.zeros((n_nodes, D), dtype=np.float32)
    rows_all = []
    for k in range(N_CORES):
        arr = np.asarray(results[k]["out"])  # [NBt//OGRP, P, OGRP*NH*D]
        rows = (
            arr.astype(np.float32)
            .reshape(NBt // OGRP, P, OGRP, NH, D)
            .transpose(0, 2, 3, 1, 4)
            .reshape(NBt * SW, D)
        )
        rows_all.append(rows)
    rows_all = np.concatenate(rows_all, axis=0)
    ids = slot_ids.reshape(-1)
    valid = ids >= 0
    iv = ids[valid]
    rv = rows_all[valid]
    order = np.argsort(iv, kind="stable")
    iv, rv = iv[order], rv[order]
    starts = np.concatenate([[0], np.nonzero(iv[1:] != iv[:-1])[0] + 1])
    sums = np.add.reduceat(rv, starts, axis=0)
    out[iv[starts]] = sums
    return out


_PROGRAM_CACHE = {}


def kernel(x, weight, edge_index, edge_vals, num_nodes):
    from concourse.bass_utils import run_bass_kernel_spmd

    N = int(num_nodes)
    in_maps, slot_ids, chunk_rows, B_g = make_in_maps(
        x, weight, edge_index, edge_vals, num_nodes
    )

    key = (tuple(chunk_rows), tuple(B_g), N)
    if key not in _PROGRAM_CACHE:
        _PROGRAM_CACHE[key] = _build_program(chunk_rows, B_g, N)
    nc = _PROGRAM_CACHE[key]

    res = run_bass_kernel_spmd(nc, in_maps, list(range(N_CORES)))
    return combine_output(res.results, slot_ids, B_g, N)
